# revision 17
# baseline (speedup 1.0000x reference)
"""ContxE-style temporal KG embedding scoring kernel for Trainium2 (Bass/Tile).

Contract: kernel(**inputs) takes FULL unsharded numpy inputs and returns the
FULL [B] float32 output. Internally shards the batch across 8 NeuronCores
(data-parallel, tables replicated) and runs a Bass/Tile kernel via
run_bass_kernel_spmd.

Math (per batch element b, window W=5, D=512):
  idx[b,w] = d[b]-(4-w), clamped: negatives -> 365
  c/s[b,w,:] = cos/sin(time_table[idx[b,w]])
  h_real = hr*c - hi*s ; h_img = hr*s + hi*c   (same for t)
  4 attention softmaxes over W of <r, rotated>, then weighted sums,
  out = sum|y_r + rr - z_r| + sum|y_i + ri + z_i|

Device-side strategy (per core, B_loc=2048, 4 blocks of 512):
  The two per-element contractions run on the TensorEngine against the
  (small, replicated) extended cos|sin table rather than on DVE:
    phase A:  V[i,b] = <U_ty[b,:], cs_ext[i,:]> for ALL 384 padded table
              rows i as a matmul (stationary = transposed cs table,
              moving = U^T).  The 5 window logits are V[day..day+4, b].
    masked exp:  E = exp(V) * mask  (mask[i,b] = day_b <= i <= day_b+4,
              host-precomputed) gives softmax numerators already in
              [i, b] layout -- no gather/scatter.
    phase B:  CSS[b,:] = E.T @ cs_ext (+ ones column for the softmax
              denominator D), landing back in [b, d] layout; the 1/D
              normalization is folded into the ACT PSUM->SBUF copy as a
              per-partition scale.
  U^T ([d', b] layout) is built from embedding factors transposed via a
  DRAM round-trip with xbar DMA-transpose. Embedding gathers use paired
  [real|img] bf16 rows (one 2KB indirect-DMA row per entity).
"""

import sys

if "/opt/trn_rl_repo" not in sys.path:
    sys.path.insert(0, "/opt/trn_rl_repo")

import numpy as np
import ml_dtypes

import concourse.bass as bass
import concourse.bacc as bacc
import concourse.tile as tile
from concourse import mybir
from concourse.bass_utils import run_bass_kernel_spmd

N_CORES = 8
B = 16384
BL = B // N_CORES          # 2048 per core
P = 128
T = BL // P                # 16 tiles of 128 per core
D = 512
DD = 2 * D                 # 1024 (cos|sin pair width)
W = 5
N_ENTITY = 100000
N_RELATION = 256
PAD_DAY = 365
NI = 384                   # padded extended-table rows (370 used)
IC = NI // P               # 3 i-chunks
JC = DD // P               # 8 d'-chunks
BLK = 512                  # batch block
NBLK = BL // BLK           # 4
SUB = BLK // P             # 4 sub-tiles of 128 per block

F32 = mybir.dt.float32
BF16 = mybir.dt.bfloat16
I32 = mybir.dt.int32

# Static active-chunk schedule for the day-sorted batch.  With d_i ~
# uniform[0,366) and 2048 elements per core sorted by day, sub-tile s
# (128 elements) spans days ~[22.875*s, 22.875*(s+1)] with quantile
# sigma ~4 days; the sets below include >=5-sigma margins.  A host-side
# check patches the (essentially impossible) violating elements.
SB_SETS = [
    (0,), (0,), (0,), (0,),
    (0, 1), (0, 1), (0, 1), (0, 1),
    (1,), (1,), (1, 2), (1, 2),
    (1, 2), (2,), (2,), (2,),
]
KA_SETS = [
    tuple(sorted(set(k for s in range(b * SUB, (b + 1) * SUB)
                 for k in SB_SETS[s]))) for b in range(NBLK)
]

AF = mybir.ActivationFunctionType
OP = mybir.AluOpType


from concourse._compat import with_exitstack


@with_exitstack
def _emit(ctx, tc, outs, ins):
    """Emit the per-core program. outs/ins are dicts of DRAM APs."""
    nc = tc.nc
    embEp = ins["embEp"]      # [N_ENTITY, 1024] bf16  ([real|img] paired rows)
    embRp = ins["embRp"]      # [N_RELATION, 1024] bf16
    csF_d = ins["csF"]        # [128, IC*DD]  bf16  forward ext table, chunked
    csT_d = ins["csT"]        # [128, JC*NI]  bf16  transposed ext table
    mask_d = ins["maskT"]     # [128, IC*BL]  bf16  window mask [i, b]
    ht_idx = ins["ht_idx"]    # [P, T*2] i32  (h, t per tile col)
    r_idx = ins["r_idx"]      # [P, T]   i32
    out = outs["out"]         # [P, T] f32

    singles = ctx.enter_context(tc.tile_pool(name="singles", bufs=1))
    gpool = ctx.enter_context(tc.tile_pool(name="g", bufs=2))
    tpool = ctx.enter_context(tc.tile_pool(name="t", bufs=2))
    upool = ctx.enter_context(tc.tile_pool(name="u", bufs=2))
    epool = ctx.enter_context(tc.tile_pool(name="e", bufs=2))
    apool = ctx.enter_context(tc.tile_pool(name="a", bufs=1))
    wpool = ctx.enter_context(tc.tile_pool(name="w", bufs=1))
    vpsum = ctx.enter_context(tc.tile_pool(name="vps", bufs=1, space="PSUM"))
    cpsum = ctx.enter_context(tc.tile_pool(name="cps", bufs=2, space="PSUM"))
    dpsum = ctx.enter_context(tc.tile_pool(name="dps", bufs=2, space="PSUM"))
    dram = ctx.enter_context(tc.tile_pool(name="dram", bufs=4, space="DRAM"))

    # --- resident tables / indices ---
    csF = singles.tile([P, IC, DD], BF16)    # csF[p,k,:] = cs_pad[k*128+p,:]
    csT = singles.tile([P, JC, NI], BF16)    # csT[p,j,i] = cs_pad[i,j*128+p]
    mask = singles.tile([P, IC, BL], BF16)   # mask[p,k,b]
    sb_ht = singles.tile([P, T * 2], I32)
    sb_r = singles.tile([P, T], I32)
    ones = singles.tile([P, 1], BF16)
    out_acc = singles.tile([P, T], F32)
    nc.sync.dma_start(csF[:], csF_d.rearrange("p (k n) -> p k n", k=IC))
    nc.sync.dma_start(csT[:], csT_d.rearrange("p (j n) -> p j n", j=JC))
    nc.sync.dma_start(mask[:], mask_d.rearrange("p (k n) -> p k n", k=IC))
    nc.sync.dma_start(sb_ht[:], ht_idx[:])
    nc.sync.dma_start(sb_r[:], r_idx[:])
    nc.vector.memset(ones[:], 1.0)

    for blk in range(NBLK):
        # ---- gathers: paired [real|img] rows ----
        hp = gpool.tile([P, SUB, DD], BF16, tag="hp")
        tp = gpool.tile([P, SUB, DD], BF16, tag="tp")
        rp = gpool.tile([P, SUB, DD], BF16, tag="rp")
        for st in range(SUB):
            t_g = blk * SUB + st
            nc.gpsimd.indirect_dma_start(
                out=hp[:, st, :], out_offset=None, in_=embEp[:],
                in_offset=bass.IndirectOffsetOnAxis(
                    ap=sb_ht[:, 2 * t_g: 2 * t_g + 1], axis=0))
            nc.gpsimd.indirect_dma_start(
                out=tp[:, st, :], out_offset=None, in_=embEp[:],
                in_offset=bass.IndirectOffsetOnAxis(
                    ap=sb_ht[:, 2 * t_g + 1: 2 * t_g + 2], axis=0))
            nc.gpsimd.indirect_dma_start(
                out=rp[:, st, :], out_offset=None, in_=embRp[:],
                in_offset=bass.IndirectOffsetOnAxis(
                    ap=sb_r[:, t_g: t_g + 1], axis=0))

        # ---- transpose factors via DRAM round-trip + ONE xbar transpose ----
        scr = dram.tile([BLK, 3 * DD], BF16, tag="scr")
        nc.sync.dma_start(
            scr[:, 0:DD].rearrange("(st p) d -> p st d", p=P), hp[:])
        nc.sync.dma_start(
            scr[:, DD:2 * DD].rearrange("(st p) d -> p st d", p=P), tp[:])
        nc.sync.dma_start(
            scr[:, 2 * DD:3 * DD].rearrange("(st p) d -> p st d", p=P), rp[:])
        xT = tpool.tile([P, 3 * JC, BLK], BF16, tag="xT")
        nc.sync.dma_start_transpose(xT[:], scr[:])
        hT = xT[:, 0:JC, :]
        tT = xT[:, JC:2 * JC, :]
        rT = xT[:, 2 * JC:3 * JC, :]

        # ---- U^T build in cos/sin j-halves (double-buffered) ----
        # ty0 = [rr*hr | -rr*hi], ty1 = [ri*hi | ri*hr],
        # ty2 = [rr*tr | -rr*ti], ty3 = [ri*ti | ri*tr]
        uh0 = upool.tile([P, 4, JC // 2, BLK], BF16, tag="U")   # j 0..3 (cos)
        uh1 = upool.tile([P, 4, JC // 2, BLK], BF16, tag="U")   # j 4..7 (sin)
        nr = wpool.tile([P, JC // 2, BLK], BF16, tag="nr")
        nc.vector.tensor_scalar(out=nr[:], in0=rT[:, 0:4, :], scalar1=-1.0,
                                scalar2=None, op0=OP.mult)
        nc.vector.tensor_tensor(out=uh0[:, 0], in0=rT[:, 0:4, :],
                                in1=hT[:, 0:4, :], op=OP.mult)
        nc.vector.tensor_tensor(out=uh0[:, 1], in0=rT[:, 4:8, :],
                                in1=hT[:, 4:8, :], op=OP.mult)
        nc.vector.tensor_tensor(out=uh0[:, 2], in0=rT[:, 0:4, :],
                                in1=tT[:, 0:4, :], op=OP.mult)
        nc.vector.tensor_tensor(out=uh0[:, 3], in0=rT[:, 4:8, :],
                                in1=tT[:, 4:8, :], op=OP.mult)
        nc.vector.tensor_tensor(out=uh1[:, 0], in0=nr[:],
                                in1=hT[:, 4:8, :], op=OP.mult)
        nc.vector.tensor_tensor(out=uh1[:, 1], in0=rT[:, 4:8, :],
                                in1=hT[:, 0:4, :], op=OP.mult)
        nc.vector.tensor_tensor(out=uh1[:, 2], in0=nr[:],
                                in1=tT[:, 4:8, :], op=OP.mult)
        nc.vector.tensor_tensor(out=uh1[:, 3], in0=rT[:, 4:8, :],
                                in1=tT[:, 0:4, :], op=OP.mult)

        # ---- phase A: V[i,b] per active i-chunk, then E = exp(V)*mask ----
        # E slot q holds chunk KA_SETS[blk][q]
        E = epool.tile([P, 4, 2, BLK], BF16, tag="E")
        for q, k in enumerate(KA_SETS[blk]):
            for tp2 in range(2):           # ty pairs share stationary loads
                vts = vpsum.tile([P, 2, BLK], F32, tag="vts")
                for j in range(JC):
                    lhsT = csT[:, j, k * P:(k + 1) * P]
                    uh = uh0 if j < JC // 2 else uh1
                    for tyh in range(2):
                        ty = tp2 * 2 + tyh
                        nc.tensor.matmul(
                            vts[:, tyh, :], lhsT=lhsT,
                            rhs=uh[:, ty, j % (JC // 2), :],
                            start=(j == 0), stop=(j == JC - 1))
                for tyh in range(2):
                    ty = tp2 * 2 + tyh
                    nc.scalar.activation(E[:, ty, q, :], vts[:, tyh, :], AF.Exp)
                    nc.vector.tensor_tensor(
                        out=E[:, ty, q, :], in0=E[:, ty, q, :],
                        in1=mask[:, k, blk * BLK:(blk + 1) * BLK], op=OP.mult)

        # ---- phase B + C per 128-row sub-tile ----
        for s in range(SUB):
            bs = slice(s * P, (s + 1) * P)
            dps = dpsum.tile([P, 4], F32, tag="dps")
            A = apool.tile([P, 4, DD], BF16, tag="A")
            rd = wpool.tile([P, 4], F32, tag="rd")
            ks = SB_SETS[blk * SUB + s]
            css = []
            for ty in range(4):
                cps = cpsum.tile([P, DD], F32, tag="cps")
                swap = ty in (1, 3)   # store CSS as [As|Ac] for img types
                for k in ks:
                    st_, sp_ = (k == ks[0]), (k == ks[-1])
                    lhsT = E[:, ty, KA_SETS[blk].index(k), bs]
                    lo = csF[:, k, D:DD] if swap else csF[:, k, 0:D]
                    hi = csF[:, k, 0:D] if swap else csF[:, k, D:DD]
                    nc.tensor.matmul(cps[:, 0:D], lhsT=lhsT, rhs=lo,
                                     start=st_, stop=sp_)
                    nc.tensor.matmul(cps[:, D:DD], lhsT=lhsT, rhs=hi,
                                     start=st_, stop=sp_)
                    nc.tensor.matmul(dps[:, ty:ty + 1], lhsT=lhsT,
                                     rhs=ones[:, 0:1], start=st_, stop=sp_)
                css.append(cps)
            nc.vector.reciprocal(rd[:], dps[:])
            for ty in range(4):
                nc.scalar.activation(A[:, ty, :], css[ty][:], AF.Copy,
                                     scale=rd[:, ty:ty + 1])

            # recombine in [b, d] layout
            # G = [hr*A0c | hi*A0s | hr*A1s | hi*A1c], H same with t/A2/A3
            G = wpool.tile([P, 2, DD], BF16, tag="G")
            H = wpool.tile([P, 2, DD], BF16, tag="H")
            nc.vector.tensor_tensor(
                out=G[:], in0=hp[:, s, None, :].broadcast_to([P, 2, DD]),
                in1=A[:, 0:2, :], op=OP.mult)
            nc.gpsimd.tensor_tensor(
                out=H[:], in0=tp[:, s, None, :].broadcast_to([P, 2, DD]),
                in1=A[:, 2:4, :], op=OP.mult)
            yr = wpool.tile([P, D], BF16, tag="yr")
            yi = wpool.tile([P, D], BF16, tag="yi")
            zr = wpool.tile([P, D], BF16, tag="zr")
            zi = wpool.tile([P, D], BF16, tag="zi")
            nc.vector.tensor_tensor(out=yr[:], in0=G[:, 0, 0:D],
                                    in1=G[:, 0, D:DD], op=OP.subtract)
            nc.vector.tensor_tensor(out=yi[:], in0=G[:, 1, 0:D],
                                    in1=G[:, 1, D:DD], op=OP.add)
            nc.vector.tensor_tensor(out=zr[:], in0=H[:, 0, 0:D],
                                    in1=H[:, 0, D:DD], op=OP.subtract)
            nc.vector.tensor_tensor(out=zi[:], in0=H[:, 1, 0:D],
                                    in1=H[:, 1, D:DD], op=OP.add)
            f1 = wpool.tile([P, D], BF16, tag="f1")
            f2 = wpool.tile([P, D], BF16, tag="f2")
            o_r = wpool.tile([P, 1], F32, tag="o_r")
            o_i = wpool.tile([P, 1], F32, tag="o_i")
            dm = wpool.tile([P, D], BF16, tag="dm")
            # f1 = yr - zr + rr ; f2 = yi + zi + ri
            nc.vector.scalar_tensor_tensor(
                out=f1[:], in0=zr[:], scalar=-1.0, in1=yr[:],
                op0=OP.mult, op1=OP.add)
            nc.vector.tensor_tensor(out=f1[:], in0=f1[:], in1=rp[:, s, 0:D],
                                    op=OP.add)
            nc.vector.tensor_tensor(out=f2[:], in0=yi[:], in1=zi[:], op=OP.add)
            nc.vector.tensor_tensor(out=f2[:], in0=f2[:], in1=rp[:, s, D:DD],
                                    op=OP.add)
            nc.scalar.activation(dm[:], f1[:], AF.Abs, accum_out=o_r[:])
            nc.scalar.activation(dm[:], f2[:], AF.Abs, accum_out=o_i[:])
            nc.vector.tensor_tensor(
                out=out_acc[:, blk * SUB + s: blk * SUB + s + 1],
                in0=o_r[:], in1=o_i[:], op=OP.add)

    nc.sync.dma_start(out[:], out_acc[:])


def _host_prep(h_i, t_i, r_i, d_i, emb_E_real, emb_E_img, emb_R_real,
               emb_R_img, time_table):
    """Host-side layout prep (table packing / index manipulation only)."""
    embEp = np.ascontiguousarray(
        np.concatenate([emb_E_real, emb_E_img], axis=1)).astype(
            ml_dtypes.bfloat16)                       # [N_ENTITY, 1024]
    embRp = np.ascontiguousarray(
        np.concatenate([emb_R_real, emb_R_img], axis=1)).astype(
            ml_dtypes.bfloat16)                       # [N_RELATION, 1024]

    tt = np.asarray(time_table, dtype=np.float32)
    cs = np.concatenate([np.cos(tt), np.sin(tt)], axis=1)  # [367, 1024]
    cs_pad = np.zeros((NI, DD), np.float32)
    cs_pad[0:4] = cs[PAD_DAY]          # prefix rows implement neg-idx clamp
    cs_pad[4:370] = cs[0:366]
    # forward table, chunked for SBUF [128, IC, DD]
    csF = np.ascontiguousarray(
        cs_pad.reshape(IC, P, DD).transpose(1, 0, 2).reshape(P, IC * DD)
    ).astype(ml_dtypes.bfloat16)
    # transposed table for SBUF [128, JC, NI]: csT[p, j, i] = cs_pad[i, j*128+p]
    csT = np.ascontiguousarray(
        cs_pad.T.reshape(JC, P, NI).transpose(1, 0, 2).reshape(P, JC * NI)
    ).astype(ml_dtypes.bfloat16)

    d = np.asarray(d_i, dtype=np.int64)
    i_grid = np.arange(NI, dtype=np.int64)[:, None]   # [NI, 1]

    ht = np.stack([h_i, t_i], axis=1).astype(np.int32)    # [B, 2]
    rx = np.asarray(r_i, dtype=np.int32).reshape(B, 1)

    def tileize(a):
        C = a.shape[1]
        return np.ascontiguousarray(
            a.reshape(T, P, C).transpose(1, 0, 2).reshape(P, T * C))

    in_maps = []
    perms = []
    fallback = []          # original global indices needing host fixup
    for core in range(N_CORES):
        sl = slice(core * BL, (core + 1) * BL)
        perm = np.argsort(d[sl], kind="stable")        # sorted-by-day order
        perms.append(perm)
        dl = d[sl][perm]                               # [BL] sorted
        # safety check: each sub-tile's window rows must fit its static
        # chunk set; collect violating elements for exact host fixup
        ds = dl.reshape(T, P)
        for s in range(T):
            lo, hi = int(ds[s].min()), int(ds[s].max()) + 4
            ok = np.zeros(NI, bool)
            for k in SB_SETS[s]:
                ok[k * P:(k + 1) * P] = True
            if not ok[lo:hi + 1].all():
                bad = np.arange(s * P, (s + 1) * P)
                fallback.extend(core * BL + perm[bad])
        m = ((i_grid >= dl[None, :]) & (i_grid <= dl[None, :] + 4))  # [NI, BL]
        maskT = np.ascontiguousarray(
            m.reshape(IC, P, BL).transpose(1, 0, 2).reshape(P, IC * BL)
        ).astype(ml_dtypes.bfloat16)
        in_maps.append(dict(
            embEp=embEp,
            embRp=embRp,
            csF=csF,
            csT=csT,
            maskT=maskT,
            ht_idx=tileize(ht[sl][perm]),
            r_idx=tileize(rx[sl][perm]),
        ))
    return in_maps, perms, np.asarray(fallback, dtype=np.int64)


def _reference_np(h_i, t_i, r_i, d_i, eR, eI, rR, rI, tt):
    """Exact numpy replica of the reference for rare host fixups."""
    n_day = tt.shape[0] - 2
    idx = d_i[:, None] - np.arange(W - 1, -1, -1)[None, :]
    idx = np.where(idx >= 0, idx, n_day)
    dl = tt[idx]
    s_, c_ = np.sin(dl), np.cos(dl)
    hr, hi = eR[h_i][:, None, :], eI[h_i][:, None, :]
    tr, ti = eR[t_i][:, None, :], eI[t_i][:, None, :]
    h_re, h_im = hr * c_ - hi * s_, hr * s_ + hi * c_
    t_re, t_im = tr * c_ - ti * s_, tr * s_ + ti * c_
    rr, ri = rR[r_i], rI[r_i]

    def soft(lg):
        e = np.exp(lg - lg.max(axis=1, keepdims=True))
        return (e / e.sum(axis=1, keepdims=True))[..., None]

    a_r = soft(np.einsum("bd,bwd->bw", rr, h_re))
    a_i = soft(np.einsum("bd,bwd->bw", ri, h_im))
    b_r = soft(np.einsum("bd,bwd->bw", rr, t_re))
    b_i = soft(np.einsum("bd,bwd->bw", ri, t_im))
    y_r = (a_r * h_re).sum(1)
    y_i = (a_i * h_im).sum(1)
    z_r = (b_r * t_re).sum(1)
    z_i = (b_i * t_im).sum(1)
    return (np.abs(y_r + rr - z_r).sum(1)
            + np.abs(y_i + ri + z_i).sum(1)).astype(np.float32)


def build_nc():
    nc = bacc.Bacc(
        "TRN2",
        target_bir_lowering=False,
        debug=False,
        enable_asserts=False,
        num_devices=N_CORES,
    )
    ins = dict(
        embEp=nc.dram_tensor("embEp", [N_ENTITY, DD], BF16,
                             kind="ExternalInput").ap(),
        embRp=nc.dram_tensor("embRp", [N_RELATION, DD], BF16,
                             kind="ExternalInput").ap(),
        csF=nc.dram_tensor("csF", [P, IC * DD], BF16,
                           kind="ExternalInput").ap(),
        csT=nc.dram_tensor("csT", [P, JC * NI], BF16,
                           kind="ExternalInput").ap(),
        maskT=nc.dram_tensor("maskT", [P, IC * BL], BF16,
                             kind="ExternalInput").ap(),
        ht_idx=nc.dram_tensor("ht_idx", [P, T * 2], I32,
                              kind="ExternalInput").ap(),
        r_idx=nc.dram_tensor("r_idx", [P, T], I32,
                             kind="ExternalInput").ap(),
    )
    outs = dict(
        out=nc.dram_tensor("out", [P, T], F32, kind="ExternalOutput").ap(),
    )
    with tile.TileContext(nc) as tc:
        _emit(tc, outs, ins)
    nc.compile()
    return nc


_NC_CACHE = {}


def kernel(h_i, t_i, r_i, d_i, emb_E_real, emb_E_img, emb_R_real, emb_R_img,
           time_table, _want_results=False, _trace=False):
    in_maps, perms, fallback = _host_prep(
        h_i, t_i, r_i, d_i, emb_E_real, emb_E_img, emb_R_real, emb_R_img,
        time_table)
    if "nc" not in _NC_CACHE:
        _NC_CACHE["nc"] = build_nc()
    nc = _NC_CACHE["nc"]
    res = run_bass_kernel_spmd(
        nc, in_maps, core_ids=list(range(N_CORES)), trace=_trace)
    out = np.empty((B,), np.float32)
    for core in range(N_CORES):
        o = np.asarray(res.results[core]["out"]).T.reshape(BL)  # sorted order
        out[core * BL + perms[core]] = o
    if len(fallback):
        f = np.asarray(fallback)
        out[f] = _reference_np(
            np.asarray(h_i)[f], np.asarray(t_i)[f], np.asarray(r_i)[f],
            np.asarray(d_i)[f], np.asarray(emb_E_real),
            np.asarray(emb_E_img), np.asarray(emb_R_real),
            np.asarray(emb_R_img), np.asarray(time_table, dtype=np.float32))
    if _want_results:
        return out, res
    return out


# revision 18
# speedup vs baseline: 1.1617x; 1.1617x over previous
"""ContxE-style temporal KG embedding scoring kernel for Trainium2 (Bass/Tile).

Contract: kernel(**inputs) takes FULL unsharded numpy inputs and returns the
FULL [B] float32 output. Internally shards the batch across 8 NeuronCores
(data-parallel, tables replicated) and runs a Bass/Tile kernel via
run_bass_kernel_spmd.

Math (per batch element b, window W=5, D=512):
  idx[b,w] = d[b]-(4-w), clamped: negatives -> 365
  c/s[b,w,:] = cos/sin(time_table[idx[b,w]])
  h_real = hr*c - hi*s ; h_img = hr*s + hi*c   (same for t)
  4 attention softmaxes over W of <r, rotated>, then weighted sums,
  out = sum|y_r + rr - z_r| + sum|y_i + ri + z_i|

Device-side strategy (per core, B_loc=2048, 4 blocks of 512):
  The two per-element contractions run on the TensorEngine against the
  (small, replicated) extended cos|sin table rather than on DVE:
    phase A:  V[i,b] = <U_ty[b,:], cs_ext[i,:]> for ALL 384 padded table
              rows i as a matmul (stationary = transposed cs table,
              moving = U^T).  The 5 window logits are V[day..day+4, b].
    masked exp:  E = exp(V) * mask  (mask[i,b] = day_b <= i <= day_b+4,
              host-precomputed) gives softmax numerators already in
              [i, b] layout -- no gather/scatter.
    phase B:  CSS[b,:] = E.T @ cs_ext (+ ones column for the softmax
              denominator D), landing back in [b, d] layout; the 1/D
              normalization is folded into the ACT PSUM->SBUF copy as a
              per-partition scale.
  U^T ([d', b] layout) is built from embedding factors transposed via a
  DRAM round-trip with xbar DMA-transpose. Embedding gathers use paired
  [real|img] bf16 rows (one 2KB indirect-DMA row per entity).
"""

import sys

if "/opt/trn_rl_repo" not in sys.path:
    sys.path.insert(0, "/opt/trn_rl_repo")

import numpy as np
import ml_dtypes

import concourse.bass as bass
import concourse.bacc as bacc
import concourse.tile as tile
from concourse import mybir
from concourse.bass_utils import run_bass_kernel_spmd

N_CORES = 8
B = 16384
BL = B // N_CORES          # 2048 per core
P = 128
T = BL // P                # 16 tiles of 128 per core
D = 512
DD = 2 * D                 # 1024 (cos|sin pair width)
W = 5
N_ENTITY = 100000
N_RELATION = 256
PAD_DAY = 365
NI = 384                   # padded extended-table rows (370 used)
IC = NI // P               # 3 i-chunks
JC = DD // P               # 8 d'-chunks
BLK = 512                  # batch block
NBLK = BL // BLK           # 4
SUB = BLK // P             # 4 sub-tiles of 128 per block

F32 = mybir.dt.float32
BF16 = mybir.dt.bfloat16
I32 = mybir.dt.int32

# Static active-chunk schedule for the day-sorted batch.  With d_i ~
# uniform[0,366) and 2048 elements per core sorted by day, sub-tile s
# (128 elements) spans days ~[22.875*s, 22.875*(s+1)] with quantile
# sigma ~4 days; the sets below include >=5-sigma margins.  A host-side
# check patches the (essentially impossible) violating elements.
SB_SETS = [
    (0,), (0,), (0,), (0,),
    (0, 1), (0, 1), (0, 1), (0, 1),
    (1,), (1,), (1, 2), (1, 2),
    (1, 2), (2,), (2,), (2,),
]
KA_SETS = [
    tuple(sorted(set(k for s in range(b * SUB, (b + 1) * SUB)
                 for k in SB_SETS[s]))) for b in range(NBLK)
]

AF = mybir.ActivationFunctionType
OP = mybir.AluOpType


from concourse._compat import with_exitstack


@with_exitstack
def _emit(ctx, tc, outs, ins):
    """Emit the per-core program. outs/ins are dicts of DRAM APs."""
    nc = tc.nc
    embEp = ins["embEp"]      # [N_ENTITY, 1024] bf16  ([real|img] paired rows)
    embRp = ins["embRp"]      # [N_RELATION, 1024] bf16
    csF_d = ins["csF"]        # [128, IC*DD]  bf16  forward ext table, chunked
    csT_d = ins["csT"]        # [128, JC*NI]  bf16  transposed ext table
    mask_d = ins["maskT"]     # [128, IC*BL]  bf16  window mask [i, b]
    ht_idx = ins["ht_idx"]    # [P, T*2] i32  (h, t per tile col)
    r_idx = ins["r_idx"]      # [P, T]   i32
    out = outs["out"]         # [P, T] f32

    singles = ctx.enter_context(tc.tile_pool(name="singles", bufs=1))
    gpool = ctx.enter_context(tc.tile_pool(name="g", bufs=2))
    tpool = ctx.enter_context(tc.tile_pool(name="t", bufs=2))
    upool = ctx.enter_context(tc.tile_pool(name="u", bufs=2))
    epool = ctx.enter_context(tc.tile_pool(name="e", bufs=2))
    apool = ctx.enter_context(tc.tile_pool(name="a", bufs=1))
    wpool = ctx.enter_context(tc.tile_pool(name="w", bufs=1))
    vpsum = ctx.enter_context(tc.tile_pool(name="vps", bufs=1, space="PSUM"))
    cpsum = ctx.enter_context(tc.tile_pool(name="cps", bufs=2, space="PSUM"))
    dpsum = ctx.enter_context(tc.tile_pool(name="dps", bufs=2, space="PSUM"))
    dram = ctx.enter_context(tc.tile_pool(name="dram", bufs=4, space="DRAM"))

    # --- resident tables / indices ---
    csF = singles.tile([P, IC, DD], BF16)    # csF[p,k,:] = cs_pad[k*128+p,:]
    csT = singles.tile([P, JC, NI], BF16)    # csT[p,j,i] = cs_pad[i,j*128+p]
    mask = singles.tile([P, IC, BL], BF16)   # mask[p,k,b]
    sb_ht = singles.tile([P, T * 2], I32)
    sb_r = singles.tile([P, T], I32)
    ones = singles.tile([P, 1], BF16)
    out_acc = singles.tile([P, T], F32)
    nc.sync.dma_start(csF[:], csF_d.rearrange("p (k n) -> p k n", k=IC))
    nc.sync.dma_start(csT[:], csT_d.rearrange("p (j n) -> p j n", j=JC))
    nc.sync.dma_start(mask[:], mask_d.rearrange("p (k n) -> p k n", k=IC))
    nc.sync.dma_start(sb_ht[:], ht_idx[:])
    nc.sync.dma_start(sb_r[:], r_idx[:])
    nc.vector.memset(ones[:], 1.0)

    for blk in range(NBLK):
        # ---- gathers: paired [real|img] rows ----
        hp = gpool.tile([P, SUB, DD], BF16, tag="hp")
        tp = gpool.tile([P, SUB, DD], BF16, tag="tp")
        rp = gpool.tile([P, SUB, DD], BF16, tag="rp")
        for st in range(SUB):
            t_g = blk * SUB + st
            nc.gpsimd.indirect_dma_start(
                out=hp[:, st, :], out_offset=None, in_=embEp[:],
                in_offset=bass.IndirectOffsetOnAxis(
                    ap=sb_ht[:, 2 * t_g: 2 * t_g + 1], axis=0))
            nc.gpsimd.indirect_dma_start(
                out=tp[:, st, :], out_offset=None, in_=embEp[:],
                in_offset=bass.IndirectOffsetOnAxis(
                    ap=sb_ht[:, 2 * t_g + 1: 2 * t_g + 2], axis=0))
            nc.gpsimd.indirect_dma_start(
                out=rp[:, st, :], out_offset=None, in_=embRp[:],
                in_offset=bass.IndirectOffsetOnAxis(
                    ap=sb_r[:, t_g: t_g + 1], axis=0))

        # ---- transpose factors via DRAM round-trip + ONE xbar transpose ----
        scr = dram.tile([BLK, 3 * DD], BF16, tag="scr")
        nc.sync.dma_start(
            scr[:, 0:DD].rearrange("(st p) d -> p st d", p=P), hp[:])
        nc.sync.dma_start(
            scr[:, DD:2 * DD].rearrange("(st p) d -> p st d", p=P), tp[:])
        nc.sync.dma_start(
            scr[:, 2 * DD:3 * DD].rearrange("(st p) d -> p st d", p=P), rp[:])
        xT = tpool.tile([P, 3 * JC, BLK], BF16, tag="xT")
        nc.sync.dma_start_transpose(xT[:], scr[:])
        hT = xT[:, 0:JC, :]
        tT = xT[:, JC:2 * JC, :]
        rT = xT[:, 2 * JC:3 * JC, :]

        # ---- U^T build in cos/sin j-halves (double-buffered) ----
        # ty0 = [rr*hr | -rr*hi], ty1 = [ri*hi | ri*hr],
        # ty2 = [rr*tr | -rr*ti], ty3 = [ri*ti | ri*tr]
        uh0 = upool.tile([P, 4, JC // 2, BLK], BF16, tag="U")   # j 0..3 (cos)
        uh1 = upool.tile([P, 4, JC // 2, BLK], BF16, tag="U")   # j 4..7 (sin)
        nr = wpool.tile([P, JC // 2, BLK], BF16, tag="nr")
        nc.vector.tensor_scalar(out=nr[:], in0=rT[:, 0:4, :], scalar1=-1.0,
                                scalar2=None, op0=OP.mult)
        nc.vector.tensor_tensor(out=uh0[:, 0], in0=rT[:, 0:4, :],
                                in1=hT[:, 0:4, :], op=OP.mult)
        nc.vector.tensor_tensor(out=uh0[:, 1], in0=rT[:, 4:8, :],
                                in1=hT[:, 4:8, :], op=OP.mult)
        nc.vector.tensor_tensor(out=uh0[:, 2], in0=rT[:, 0:4, :],
                                in1=tT[:, 0:4, :], op=OP.mult)
        nc.vector.tensor_tensor(out=uh0[:, 3], in0=rT[:, 4:8, :],
                                in1=tT[:, 4:8, :], op=OP.mult)
        nc.vector.tensor_tensor(out=uh1[:, 0], in0=nr[:],
                                in1=hT[:, 4:8, :], op=OP.mult)
        nc.vector.tensor_tensor(out=uh1[:, 1], in0=rT[:, 4:8, :],
                                in1=hT[:, 0:4, :], op=OP.mult)
        nc.vector.tensor_tensor(out=uh1[:, 2], in0=nr[:],
                                in1=tT[:, 4:8, :], op=OP.mult)
        nc.vector.tensor_tensor(out=uh1[:, 3], in0=rT[:, 4:8, :],
                                in1=tT[:, 0:4, :], op=OP.mult)

        # ---- phase A: V[i,b] per active i-chunk, then E = exp(V)*mask ----
        # E slot q holds chunk KA_SETS[blk][q]
        E = epool.tile([P, 4, 2, BLK], BF16, tag="E")
        for q, k in enumerate(KA_SETS[blk]):
            for tp2 in range(2):           # ty pairs share stationary loads
                vts = vpsum.tile([P, 2, BLK], F32, tag="vts")
                for j in range(JC):
                    lhsT = csT[:, j, k * P:(k + 1) * P]
                    uh = uh0 if j < JC // 2 else uh1
                    for tyh in range(2):
                        ty = tp2 * 2 + tyh
                        nc.tensor.matmul(
                            vts[:, tyh, :], lhsT=lhsT,
                            rhs=uh[:, ty, j % (JC // 2), :],
                            start=(j == 0), stop=(j == JC - 1))
                for tyh in range(2):
                    ty = tp2 * 2 + tyh
                    nc.scalar.activation(E[:, ty, q, :], vts[:, tyh, :], AF.Exp)
                    nc.vector.tensor_tensor(
                        out=E[:, ty, q, :], in0=E[:, ty, q, :],
                        in1=mask[:, k, blk * BLK:(blk + 1) * BLK], op=OP.mult)

        # ---- phase B + C per 128-row sub-tile ----
        for s in range(SUB):
            bs = slice(s * P, (s + 1) * P)
            dps = dpsum.tile([P, 4], F32, tag="dps")
            A = apool.tile([P, 4, DD], BF16, tag="A")
            rd = wpool.tile([P, 4], F32, tag="rd")
            ks = SB_SETS[blk * SUB + s]
            css = []
            for ty in range(4):
                cps = cpsum.tile([P, DD], F32, tag="cps")
                swap = ty in (1, 3)   # store CSS as [As|Ac] for img types
                for k in ks:
                    st_, sp_ = (k == ks[0]), (k == ks[-1])
                    lhsT = E[:, ty, KA_SETS[blk].index(k), bs]
                    lo = csF[:, k, D:DD] if swap else csF[:, k, 0:D]
                    hi = csF[:, k, 0:D] if swap else csF[:, k, D:DD]
                    nc.tensor.matmul(cps[:, 0:D], lhsT=lhsT, rhs=lo,
                                     start=st_, stop=sp_)
                    nc.tensor.matmul(cps[:, D:DD], lhsT=lhsT, rhs=hi,
                                     start=st_, stop=sp_)
                    nc.tensor.matmul(dps[:, ty:ty + 1], lhsT=lhsT,
                                     rhs=ones[:, 0:1], start=st_, stop=sp_)
                css.append(cps)
            nc.vector.reciprocal(rd[:], dps[:])
            for ty in range(4):
                nc.scalar.activation(A[:, ty, :], css[ty][:], AF.Copy,
                                     scale=rd[:, ty:ty + 1])

            # recombine in [b, d] layout
            # G = [hr*A0c | hi*A0s | hr*A1s | hi*A1c], H same with t/A2/A3
            G = wpool.tile([P, 2, DD], BF16, tag="G")
            H = wpool.tile([P, 2, DD], BF16, tag="H")
            nc.vector.tensor_tensor(
                out=G[:], in0=hp[:, s, None, :].broadcast_to([P, 2, DD]),
                in1=A[:, 0:2, :], op=OP.mult)
            nc.vector.tensor_tensor(
                out=H[:], in0=tp[:, s, None, :].broadcast_to([P, 2, DD]),
                in1=A[:, 2:4, :], op=OP.mult)
            yr = wpool.tile([P, D], BF16, tag="yr")
            yi = wpool.tile([P, D], BF16, tag="yi")
            zr = wpool.tile([P, D], BF16, tag="zr")
            zi = wpool.tile([P, D], BF16, tag="zi")
            nc.vector.tensor_tensor(out=yr[:], in0=G[:, 0, 0:D],
                                    in1=G[:, 0, D:DD], op=OP.subtract)
            nc.vector.tensor_tensor(out=yi[:], in0=G[:, 1, 0:D],
                                    in1=G[:, 1, D:DD], op=OP.add)
            nc.vector.tensor_tensor(out=zr[:], in0=H[:, 0, 0:D],
                                    in1=H[:, 0, D:DD], op=OP.subtract)
            nc.vector.tensor_tensor(out=zi[:], in0=H[:, 1, 0:D],
                                    in1=H[:, 1, D:DD], op=OP.add)
            f1 = wpool.tile([P, D], BF16, tag="f1")
            f2 = wpool.tile([P, D], BF16, tag="f2")
            o_r = wpool.tile([P, 1], F32, tag="o_r")
            o_i = wpool.tile([P, 1], F32, tag="o_i")
            dm = wpool.tile([P, D], BF16, tag="dm")
            # f1 = yr - zr + rr ; f2 = yi + zi + ri
            nc.vector.scalar_tensor_tensor(
                out=f1[:], in0=zr[:], scalar=-1.0, in1=yr[:],
                op0=OP.mult, op1=OP.add)
            nc.vector.tensor_tensor(out=f1[:], in0=f1[:], in1=rp[:, s, 0:D],
                                    op=OP.add)
            nc.vector.tensor_tensor(out=f2[:], in0=yi[:], in1=zi[:], op=OP.add)
            nc.vector.tensor_tensor(out=f2[:], in0=f2[:], in1=rp[:, s, D:DD],
                                    op=OP.add)
            nc.scalar.activation(dm[:], f1[:], AF.Abs, accum_out=o_r[:])
            nc.scalar.activation(dm[:], f2[:], AF.Abs, accum_out=o_i[:])
            nc.vector.tensor_tensor(
                out=out_acc[:, blk * SUB + s: blk * SUB + s + 1],
                in0=o_r[:], in1=o_i[:], op=OP.add)

    nc.sync.dma_start(out[:], out_acc[:])


def _host_prep(h_i, t_i, r_i, d_i, emb_E_real, emb_E_img, emb_R_real,
               emb_R_img, time_table):
    """Host-side layout prep (table packing / index manipulation only)."""
    embEp = np.ascontiguousarray(
        np.concatenate([emb_E_real, emb_E_img], axis=1)).astype(
            ml_dtypes.bfloat16)                       # [N_ENTITY, 1024]
    embRp = np.ascontiguousarray(
        np.concatenate([emb_R_real, emb_R_img], axis=1)).astype(
            ml_dtypes.bfloat16)                       # [N_RELATION, 1024]

    tt = np.asarray(time_table, dtype=np.float32)
    cs = np.concatenate([np.cos(tt), np.sin(tt)], axis=1)  # [367, 1024]
    cs_pad = np.zeros((NI, DD), np.float32)
    cs_pad[0:4] = cs[PAD_DAY]          # prefix rows implement neg-idx clamp
    cs_pad[4:370] = cs[0:366]
    # forward table, chunked for SBUF [128, IC, DD]
    csF = np.ascontiguousarray(
        cs_pad.reshape(IC, P, DD).transpose(1, 0, 2).reshape(P, IC * DD)
    ).astype(ml_dtypes.bfloat16)
    # transposed table for SBUF [128, JC, NI]: csT[p, j, i] = cs_pad[i, j*128+p]
    csT = np.ascontiguousarray(
        cs_pad.T.reshape(JC, P, NI).transpose(1, 0, 2).reshape(P, JC * NI)
    ).astype(ml_dtypes.bfloat16)

    d = np.asarray(d_i, dtype=np.int64)
    i_grid = np.arange(NI, dtype=np.int64)[:, None]   # [NI, 1]

    ht = np.stack([h_i, t_i], axis=1).astype(np.int32)    # [B, 2]
    rx = np.asarray(r_i, dtype=np.int32).reshape(B, 1)

    def tileize(a):
        C = a.shape[1]
        return np.ascontiguousarray(
            a.reshape(T, P, C).transpose(1, 0, 2).reshape(P, T * C))

    in_maps = []
    perms = []
    fallback = []          # original global indices needing host fixup
    for core in range(N_CORES):
        sl = slice(core * BL, (core + 1) * BL)
        perm = np.argsort(d[sl], kind="stable")        # sorted-by-day order
        perms.append(perm)
        dl = d[sl][perm]                               # [BL] sorted
        # safety check: each sub-tile's window rows must fit its static
        # chunk set; collect violating elements for exact host fixup
        ds = dl.reshape(T, P)
        for s in range(T):
            lo, hi = int(ds[s].min()), int(ds[s].max()) + 4
            ok = np.zeros(NI, bool)
            for k in SB_SETS[s]:
                ok[k * P:(k + 1) * P] = True
            if not ok[lo:hi + 1].all():
                bad = np.arange(s * P, (s + 1) * P)
                fallback.extend(core * BL + perm[bad])
        m = ((i_grid >= dl[None, :]) & (i_grid <= dl[None, :] + 4))  # [NI, BL]
        maskT = np.ascontiguousarray(
            m.reshape(IC, P, BL).transpose(1, 0, 2).reshape(P, IC * BL)
        ).astype(ml_dtypes.bfloat16)
        in_maps.append(dict(
            embEp=embEp,
            embRp=embRp,
            csF=csF,
            csT=csT,
            maskT=maskT,
            ht_idx=tileize(ht[sl][perm]),
            r_idx=tileize(rx[sl][perm]),
        ))
    return in_maps, perms, np.asarray(fallback, dtype=np.int64)


def _reference_np(h_i, t_i, r_i, d_i, eR, eI, rR, rI, tt):
    """Exact numpy replica of the reference for rare host fixups."""
    n_day = tt.shape[0] - 2
    idx = d_i[:, None] - np.arange(W - 1, -1, -1)[None, :]
    idx = np.where(idx >= 0, idx, n_day)
    dl = tt[idx]
    s_, c_ = np.sin(dl), np.cos(dl)
    hr, hi = eR[h_i][:, None, :], eI[h_i][:, None, :]
    tr, ti = eR[t_i][:, None, :], eI[t_i][:, None, :]
    h_re, h_im = hr * c_ - hi * s_, hr * s_ + hi * c_
    t_re, t_im = tr * c_ - ti * s_, tr * s_ + ti * c_
    rr, ri = rR[r_i], rI[r_i]

    def soft(lg):
        e = np.exp(lg - lg.max(axis=1, keepdims=True))
        return (e / e.sum(axis=1, keepdims=True))[..., None]

    a_r = soft(np.einsum("bd,bwd->bw", rr, h_re))
    a_i = soft(np.einsum("bd,bwd->bw", ri, h_im))
    b_r = soft(np.einsum("bd,bwd->bw", rr, t_re))
    b_i = soft(np.einsum("bd,bwd->bw", ri, t_im))
    y_r = (a_r * h_re).sum(1)
    y_i = (a_i * h_im).sum(1)
    z_r = (b_r * t_re).sum(1)
    z_i = (b_i * t_im).sum(1)
    return (np.abs(y_r + rr - z_r).sum(1)
            + np.abs(y_i + ri + z_i).sum(1)).astype(np.float32)


def build_nc():
    nc = bacc.Bacc(
        "TRN2",
        target_bir_lowering=False,
        debug=False,
        enable_asserts=False,
        num_devices=N_CORES,
    )
    ins = dict(
        embEp=nc.dram_tensor("embEp", [N_ENTITY, DD], BF16,
                             kind="ExternalInput").ap(),
        embRp=nc.dram_tensor("embRp", [N_RELATION, DD], BF16,
                             kind="ExternalInput").ap(),
        csF=nc.dram_tensor("csF", [P, IC * DD], BF16,
                           kind="ExternalInput").ap(),
        csT=nc.dram_tensor("csT", [P, JC * NI], BF16,
                           kind="ExternalInput").ap(),
        maskT=nc.dram_tensor("maskT", [P, IC * BL], BF16,
                             kind="ExternalInput").ap(),
        ht_idx=nc.dram_tensor("ht_idx", [P, T * 2], I32,
                              kind="ExternalInput").ap(),
        r_idx=nc.dram_tensor("r_idx", [P, T], I32,
                             kind="ExternalInput").ap(),
    )
    outs = dict(
        out=nc.dram_tensor("out", [P, T], F32, kind="ExternalOutput").ap(),
    )
    with tile.TileContext(nc) as tc:
        _emit(tc, outs, ins)
    nc.compile()
    return nc


_NC_CACHE = {}


def kernel(h_i, t_i, r_i, d_i, emb_E_real, emb_E_img, emb_R_real, emb_R_img,
           time_table, _want_results=False, _trace=False):
    in_maps, perms, fallback = _host_prep(
        h_i, t_i, r_i, d_i, emb_E_real, emb_E_img, emb_R_real, emb_R_img,
        time_table)
    if "nc" not in _NC_CACHE:
        _NC_CACHE["nc"] = build_nc()
    nc = _NC_CACHE["nc"]
    res = run_bass_kernel_spmd(
        nc, in_maps, core_ids=list(range(N_CORES)), trace=_trace)
    out = np.empty((B,), np.float32)
    for core in range(N_CORES):
        o = np.asarray(res.results[core]["out"]).T.reshape(BL)  # sorted order
        out[core * BL + perms[core]] = o
    if len(fallback):
        f = np.asarray(fallback)
        out[f] = _reference_np(
            np.asarray(h_i)[f], np.asarray(t_i)[f], np.asarray(r_i)[f],
            np.asarray(d_i)[f], np.asarray(emb_E_real),
            np.asarray(emb_E_img), np.asarray(emb_R_real),
            np.asarray(emb_R_img), np.asarray(time_table, dtype=np.float32))
    if _want_results:
        return out, res
    return out


# revision 22
# speedup vs baseline: 1.1904x; 1.0248x over previous
"""ContxE-style temporal KG embedding scoring kernel for Trainium2 (Bass/Tile).

Contract: kernel(**inputs) takes FULL unsharded numpy inputs and returns the
FULL [B] float32 output. Internally shards the batch across 8 NeuronCores
(data-parallel, tables replicated) and runs a Bass/Tile kernel via
run_bass_kernel_spmd.

Math (per batch element b, window W=5, D=512):
  idx[b,w] = d[b]-(4-w), clamped: negatives -> 365
  c/s[b,w,:] = cos/sin(time_table[idx[b,w]])
  h_real = hr*c - hi*s ; h_img = hr*s + hi*c   (same for t)
  4 attention softmaxes over W of <r, rotated>, then weighted sums,
  out = sum|y_r + rr - z_r| + sum|y_i + ri + z_i|

Device-side strategy (per core, B_loc=2048, 4 blocks of 512):
  The two per-element contractions run on the TensorEngine against the
  (small, replicated) extended cos|sin table rather than on DVE:
    phase A:  V[i,b] = <U_ty[b,:], cs_ext[i,:]> for ALL 384 padded table
              rows i as a matmul (stationary = transposed cs table,
              moving = U^T).  The 5 window logits are V[day..day+4, b].
    masked exp:  E = exp(V) * mask  (mask[i,b] = day_b <= i <= day_b+4,
              host-precomputed) gives softmax numerators already in
              [i, b] layout -- no gather/scatter.
    phase B:  CSS[b,:] = E.T @ cs_ext (+ ones column for the softmax
              denominator D), landing back in [b, d] layout; the 1/D
              normalization is folded into the ACT PSUM->SBUF copy as a
              per-partition scale.
  U^T ([d', b] layout) is built from embedding factors transposed via a
  DRAM round-trip with xbar DMA-transpose. Embedding gathers use paired
  [real|img] bf16 rows (one 2KB indirect-DMA row per entity).
"""

import sys

if "/opt/trn_rl_repo" not in sys.path:
    sys.path.insert(0, "/opt/trn_rl_repo")

import numpy as np
import ml_dtypes

import concourse.bass as bass
import concourse.bacc as bacc
import concourse.tile as tile
from concourse import mybir
from concourse.bass_utils import run_bass_kernel_spmd

N_CORES = 8
B = 16384
BL = B // N_CORES          # 2048 per core
P = 128
T = BL // P                # 16 tiles of 128 per core
D = 512
DD = 2 * D                 # 1024 (cos|sin pair width)
W = 5
N_ENTITY = 100000
N_RELATION = 256
PAD_DAY = 365
NI = 384                   # padded extended-table rows (370 used)
IC = NI // P               # 3 i-chunks
JC = DD // P               # 8 d'-chunks
BLK = 512                  # batch block
NBLK = BL // BLK           # 4
SUB = BLK // P             # 4 sub-tiles of 128 per block

F32 = mybir.dt.float32
BF16 = mybir.dt.bfloat16
I32 = mybir.dt.int32

# Static active-chunk schedule for the day-sorted batch.  With d_i ~
# uniform[0,366) and 2048 elements per core sorted by day, sub-tile s
# (128 elements) spans days ~[22.875*s, 22.875*(s+1)] with quantile
# sigma ~4 days; the sets below include >=5-sigma margins.  A host-side
# check patches the (essentially impossible) violating elements.
SB_SETS = [
    (0,), (0,), (0,), (0,),
    (0, 1), (0, 1), (0, 1), (1,),
    (1,), (1,), (1, 2), (1, 2),
    (2,), (2,), (2,), (2,),
]
KA_SETS = [
    tuple(sorted(set(k for s in range(b * SUB, (b + 1) * SUB)
                 for k in SB_SETS[s]))) for b in range(NBLK)
]

AF = mybir.ActivationFunctionType
OP = mybir.AluOpType


from concourse._compat import with_exitstack


@with_exitstack
def _emit(ctx, tc, outs, ins):
    """Emit the per-core program. outs/ins are dicts of DRAM APs."""
    nc = tc.nc
    embEp = ins["embEp"]      # [N_ENTITY, 1024] bf16  ([real|img] paired rows)
    embRp = ins["embRp"]      # [N_RELATION, 1024] bf16
    csF_d = ins["csF"]        # [128, IC*DD]  bf16  forward ext table, chunked
    csT_d = ins["csT"]        # [128, JC*NI]  bf16  transposed ext table
    mask_d = ins["maskT"]     # [128, IC*BL]  bf16  window mask [i, b]
    ht_idx = ins["ht_idx"]    # [P, T*2] i32  (h, t per tile col)
    r_idx = ins["r_idx"]      # [P, T]   i32
    out = outs["out"]         # [P, T] f32

    singles = ctx.enter_context(tc.tile_pool(name="singles", bufs=1))
    gpool = ctx.enter_context(tc.tile_pool(name="g", bufs=2))
    tpool = ctx.enter_context(tc.tile_pool(name="t", bufs=2))
    upool = ctx.enter_context(tc.tile_pool(name="u", bufs=2))
    epool = ctx.enter_context(tc.tile_pool(name="e", bufs=2))
    apool = ctx.enter_context(tc.tile_pool(name="a", bufs=1))
    wpool = ctx.enter_context(tc.tile_pool(name="w", bufs=1))
    vpsum = ctx.enter_context(tc.tile_pool(name="vps", bufs=1, space="PSUM"))
    cpsum = ctx.enter_context(tc.tile_pool(name="cps", bufs=2, space="PSUM"))
    dpsum = ctx.enter_context(tc.tile_pool(name="dps", bufs=2, space="PSUM"))
    dram = ctx.enter_context(tc.tile_pool(name="dram", bufs=4, space="DRAM"))

    # --- resident tables / indices ---
    csF = singles.tile([P, IC, DD], BF16)    # csF[p,k,:] = cs_pad[k*128+p,:]
    csT = singles.tile([P, JC, NI], BF16)    # csT[p,j,i] = cs_pad[i,j*128+p]
    mask = singles.tile([P, IC, BL], BF16)   # mask[p,k,b]
    sb_ht = singles.tile([P, T * 2], I32)
    sb_r = singles.tile([P, T], I32)
    ones = singles.tile([P, 1], BF16)
    out_acc = singles.tile([P, T], F32)
    nc.sync.dma_start(csF[:], csF_d.rearrange("p (k n) -> p k n", k=IC))
    nc.sync.dma_start(csT[:], csT_d.rearrange("p (j n) -> p j n", j=JC))
    nc.sync.dma_start(mask[:], mask_d.rearrange("p (k n) -> p k n", k=IC))
    nc.sync.dma_start(sb_ht[:], ht_idx[:])
    nc.sync.dma_start(sb_r[:], r_idx[:])
    nc.vector.memset(ones[:], 1.0)

    for blk in range(NBLK):
        # ---- gathers + transpose round-trip in half-blocks of 256 ----
        # ty0 = [rr*hr | -rr*hi], ty1 = [ri*hi | ri*hr],
        # ty2 = [rr*tr | -rr*ti], ty3 = [ri*ti | ri*tr]
        HB = BLK // 2
        scr = dram.tile([BLK, 3 * DD], BF16, tag="scr")
        uh0 = upool.tile([P, 4, JC // 2, BLK], BF16, tag="U")   # j 0..3 (cos)
        uh1 = upool.tile([P, 4, JC // 2, BLK], BF16, tag="U")   # j 4..7 (sin)
        gp_halves = []
        for h in range(2):
            hp_h = gpool.tile([P, 2, DD], BF16, tag=f"hp{h}")
            tp_h = gpool.tile([P, 2, DD], BF16, tag=f"tp{h}")
            rp_h = gpool.tile([P, 2, DD], BF16, tag=f"rp{h}")
            gp_halves.append((hp_h, tp_h, rp_h))
            for st in range(2):
                t_g = blk * SUB + h * 2 + st
                nc.gpsimd.indirect_dma_start(
                    out=hp_h[:, st, :], out_offset=None, in_=embEp[:],
                    in_offset=bass.IndirectOffsetOnAxis(
                        ap=sb_ht[:, 2 * t_g: 2 * t_g + 1], axis=0))
                nc.gpsimd.indirect_dma_start(
                    out=tp_h[:, st, :], out_offset=None, in_=embEp[:],
                    in_offset=bass.IndirectOffsetOnAxis(
                        ap=sb_ht[:, 2 * t_g + 1: 2 * t_g + 2], axis=0))
                nc.gpsimd.indirect_dma_start(
                    out=rp_h[:, st, :], out_offset=None, in_=embRp[:],
                    in_offset=bass.IndirectOffsetOnAxis(
                        ap=sb_r[:, t_g: t_g + 1], axis=0))
            rows = slice(h * HB, (h + 1) * HB)
            nc.sync.dma_start(
                scr[rows, 0:DD].rearrange("(st p) d -> p st d", p=P), hp_h[:])
            nc.sync.dma_start(
                scr[rows, DD:2 * DD].rearrange("(st p) d -> p st d", p=P),
                tp_h[:])
            nc.sync.dma_start(
                scr[rows, 2 * DD:3 * DD].rearrange("(st p) d -> p st d", p=P),
                rp_h[:])
            xh = tpool.tile([P, 3 * JC, HB], BF16, tag=f"xT{h}")
            nc.sync.dma_start_transpose(xh[:], scr[rows, :])
            hT = xh[:, 0:JC, :]
            tT = xh[:, JC:2 * JC, :]
            rT = xh[:, 2 * JC:3 * JC, :]
            bsl = slice(h * HB, (h + 1) * HB)
            nr = wpool.tile([P, JC // 2, HB], BF16, tag="nr")
            nc.vector.tensor_scalar(out=nr[:], in0=rT[:, 0:4, :], scalar1=-1.0,
                                    scalar2=None, op0=OP.mult)
            nc.vector.tensor_tensor(out=uh0[:, 0, :, bsl], in0=rT[:, 0:4, :],
                                    in1=hT[:, 0:4, :], op=OP.mult)
            nc.vector.tensor_tensor(out=uh0[:, 1, :, bsl], in0=rT[:, 4:8, :],
                                    in1=hT[:, 4:8, :], op=OP.mult)
            nc.vector.tensor_tensor(out=uh0[:, 2, :, bsl], in0=rT[:, 0:4, :],
                                    in1=tT[:, 0:4, :], op=OP.mult)
            nc.vector.tensor_tensor(out=uh0[:, 3, :, bsl], in0=rT[:, 4:8, :],
                                    in1=tT[:, 4:8, :], op=OP.mult)
            nc.vector.tensor_tensor(out=uh1[:, 0, :, bsl], in0=nr[:],
                                    in1=hT[:, 4:8, :], op=OP.mult)
            nc.vector.tensor_tensor(out=uh1[:, 1, :, bsl], in0=rT[:, 4:8, :],
                                    in1=hT[:, 0:4, :], op=OP.mult)
            nc.vector.tensor_tensor(out=uh1[:, 2, :, bsl], in0=nr[:],
                                    in1=tT[:, 4:8, :], op=OP.mult)
            nc.vector.tensor_tensor(out=uh1[:, 3, :, bsl], in0=rT[:, 4:8, :],
                                    in1=tT[:, 0:4, :], op=OP.mult)

        # ---- phase A: V[i,b] per active i-chunk, then E = exp(V)*mask ----
        # E slot q holds chunk KA_SETS[blk][q]
        E = epool.tile([P, 4, 2, BLK], BF16, tag="E")
        for q, k in enumerate(KA_SETS[blk]):
            for tp2 in range(2):           # ty pairs share stationary loads
                vts = vpsum.tile([P, 2, BLK], F32, tag="vts")
                for j in range(JC):
                    lhsT = csT[:, j, k * P:(k + 1) * P]
                    uh = uh0 if j < JC // 2 else uh1
                    for tyh in range(2):
                        ty = tp2 * 2 + tyh
                        nc.tensor.matmul(
                            vts[:, tyh, :], lhsT=lhsT,
                            rhs=uh[:, ty, j % (JC // 2), :],
                            start=(j == 0), stop=(j == JC - 1))
                for tyh in range(2):
                    ty = tp2 * 2 + tyh
                    nc.scalar.activation(E[:, ty, q, :], vts[:, tyh, :], AF.Exp)
                    nc.vector.tensor_tensor(
                        out=E[:, ty, q, :], in0=E[:, ty, q, :],
                        in1=mask[:, k, blk * BLK:(blk + 1) * BLK], op=OP.mult)

        # ---- phase B + C per 128-row sub-tile ----
        for s in range(SUB):
            bs = slice(s * P, (s + 1) * P)
            dps = dpsum.tile([P, 4], F32, tag="dps")
            A = apool.tile([P, 4, DD], BF16, tag="A")
            rd = wpool.tile([P, 4], F32, tag="rd")
            ks = SB_SETS[blk * SUB + s]
            css = []
            for ty in range(4):
                cps = cpsum.tile([P, DD], F32, tag="cps")
                swap = ty in (1, 3)   # store CSS as [As|Ac] for img types
                for k in ks:
                    st_, sp_ = (k == ks[0]), (k == ks[-1])
                    lhsT = E[:, ty, KA_SETS[blk].index(k), bs]
                    lo = csF[:, k, D:DD] if swap else csF[:, k, 0:D]
                    hi = csF[:, k, 0:D] if swap else csF[:, k, D:DD]
                    nc.tensor.matmul(cps[:, 0:D], lhsT=lhsT, rhs=lo,
                                     start=st_, stop=sp_)
                    nc.tensor.matmul(cps[:, D:DD], lhsT=lhsT, rhs=hi,
                                     start=st_, stop=sp_)
                    nc.tensor.matmul(dps[:, ty:ty + 1], lhsT=lhsT,
                                     rhs=ones[:, 0:1], start=st_, stop=sp_)
                css.append(cps)
            nc.vector.reciprocal(rd[:], dps[:])
            for ty in range(4):
                nc.scalar.activation(A[:, ty, :], css[ty][:], AF.Copy,
                                     scale=rd[:, ty:ty + 1])

            # recombine in [b, d] layout
            # G = [hr*A0c | hi*A0s | hr*A1s | hi*A1c], H same with t/A2/A3
            hp_h, tp_h, rp_h = gp_halves[s // 2]
            sh = s % 2
            G = wpool.tile([P, 2, DD], BF16, tag="G")
            H = wpool.tile([P, 2, DD], BF16, tag="H")
            nc.vector.tensor_tensor(
                out=G[:], in0=hp_h[:, sh, None, :].broadcast_to([P, 2, DD]),
                in1=A[:, 0:2, :], op=OP.mult)
            nc.vector.tensor_tensor(
                out=H[:], in0=tp_h[:, sh, None, :].broadcast_to([P, 2, DD]),
                in1=A[:, 2:4, :], op=OP.mult)
            yr = wpool.tile([P, D], BF16, tag="yr")
            yi = wpool.tile([P, D], BF16, tag="yi")
            zr = wpool.tile([P, D], BF16, tag="zr")
            zi = wpool.tile([P, D], BF16, tag="zi")
            nc.vector.tensor_tensor(out=yr[:], in0=G[:, 0, 0:D],
                                    in1=G[:, 0, D:DD], op=OP.subtract)
            nc.vector.tensor_tensor(out=yi[:], in0=G[:, 1, 0:D],
                                    in1=G[:, 1, D:DD], op=OP.add)
            nc.vector.tensor_tensor(out=zr[:], in0=H[:, 0, 0:D],
                                    in1=H[:, 0, D:DD], op=OP.subtract)
            nc.vector.tensor_tensor(out=zi[:], in0=H[:, 1, 0:D],
                                    in1=H[:, 1, D:DD], op=OP.add)
            f1 = wpool.tile([P, D], BF16, tag="f1")
            f2 = wpool.tile([P, D], BF16, tag="f2")
            o_r = wpool.tile([P, 1], F32, tag="o_r")
            o_i = wpool.tile([P, 1], F32, tag="o_i")
            dm = wpool.tile([P, D], BF16, tag="dm")
            # f1 = yr - zr + rr ; f2 = yi + zi + ri
            nc.vector.scalar_tensor_tensor(
                out=f1[:], in0=zr[:], scalar=-1.0, in1=yr[:],
                op0=OP.mult, op1=OP.add)
            nc.vector.tensor_tensor(out=f1[:], in0=f1[:], in1=rp_h[:, sh, 0:D],
                                    op=OP.add)
            nc.vector.tensor_tensor(out=f2[:], in0=yi[:], in1=zi[:], op=OP.add)
            nc.vector.tensor_tensor(out=f2[:], in0=f2[:],
                                    in1=rp_h[:, sh, D:DD], op=OP.add)
            nc.scalar.activation(dm[:], f1[:], AF.Abs, accum_out=o_r[:])
            nc.scalar.activation(dm[:], f2[:], AF.Abs, accum_out=o_i[:])
            nc.vector.tensor_tensor(
                out=out_acc[:, blk * SUB + s: blk * SUB + s + 1],
                in0=o_r[:], in1=o_i[:], op=OP.add)

    nc.sync.dma_start(out[:], out_acc[:])


def _host_prep(h_i, t_i, r_i, d_i, emb_E_real, emb_E_img, emb_R_real,
               emb_R_img, time_table):
    """Host-side layout prep (table packing / index manipulation only)."""
    embEp = np.ascontiguousarray(
        np.concatenate([emb_E_real, emb_E_img], axis=1)).astype(
            ml_dtypes.bfloat16)                       # [N_ENTITY, 1024]
    embRp = np.ascontiguousarray(
        np.concatenate([emb_R_real, emb_R_img], axis=1)).astype(
            ml_dtypes.bfloat16)                       # [N_RELATION, 1024]

    tt = np.asarray(time_table, dtype=np.float32)
    cs = np.concatenate([np.cos(tt), np.sin(tt)], axis=1)  # [367, 1024]
    cs_pad = np.zeros((NI, DD), np.float32)
    cs_pad[0:4] = cs[PAD_DAY]          # prefix rows implement neg-idx clamp
    cs_pad[4:370] = cs[0:366]
    # forward table, chunked for SBUF [128, IC, DD]
    csF = np.ascontiguousarray(
        cs_pad.reshape(IC, P, DD).transpose(1, 0, 2).reshape(P, IC * DD)
    ).astype(ml_dtypes.bfloat16)
    # transposed table for SBUF [128, JC, NI]: csT[p, j, i] = cs_pad[i, j*128+p]
    csT = np.ascontiguousarray(
        cs_pad.T.reshape(JC, P, NI).transpose(1, 0, 2).reshape(P, JC * NI)
    ).astype(ml_dtypes.bfloat16)

    d = np.asarray(d_i, dtype=np.int64)
    i_grid = np.arange(NI, dtype=np.int64)[:, None]   # [NI, 1]

    ht = np.stack([h_i, t_i], axis=1).astype(np.int32)    # [B, 2]
    rx = np.asarray(r_i, dtype=np.int32).reshape(B, 1)

    def tileize(a):
        C = a.shape[1]
        return np.ascontiguousarray(
            a.reshape(T, P, C).transpose(1, 0, 2).reshape(P, T * C))

    in_maps = []
    perms = []
    fallback = []          # original global indices needing host fixup
    for core in range(N_CORES):
        sl = slice(core * BL, (core + 1) * BL)
        perm = np.argsort(d[sl], kind="stable")        # sorted-by-day order
        perms.append(perm)
        dl = d[sl][perm]                               # [BL] sorted
        # safety check: each sub-tile's window rows must fit its static
        # chunk set; collect violating elements for exact host fixup
        ds = dl.reshape(T, P)
        for s in range(T):
            lo, hi = int(ds[s].min()), int(ds[s].max()) + 4
            ok = np.zeros(NI, bool)
            for k in SB_SETS[s]:
                ok[k * P:(k + 1) * P] = True
            if not ok[lo:hi + 1].all():
                bad = np.arange(s * P, (s + 1) * P)
                fallback.extend(core * BL + perm[bad])
        m = ((i_grid >= dl[None, :]) & (i_grid <= dl[None, :] + 4))  # [NI, BL]
        maskT = np.ascontiguousarray(
            m.reshape(IC, P, BL).transpose(1, 0, 2).reshape(P, IC * BL)
        ).astype(ml_dtypes.bfloat16)
        in_maps.append(dict(
            embEp=embEp,
            embRp=embRp,
            csF=csF,
            csT=csT,
            maskT=maskT,
            ht_idx=tileize(ht[sl][perm]),
            r_idx=tileize(rx[sl][perm]),
        ))
    return in_maps, perms, np.asarray(fallback, dtype=np.int64)


def _reference_np(h_i, t_i, r_i, d_i, eR, eI, rR, rI, tt):
    """Exact numpy replica of the reference for rare host fixups."""
    n_day = tt.shape[0] - 2
    idx = d_i[:, None] - np.arange(W - 1, -1, -1)[None, :]
    idx = np.where(idx >= 0, idx, n_day)
    dl = tt[idx]
    s_, c_ = np.sin(dl), np.cos(dl)
    hr, hi = eR[h_i][:, None, :], eI[h_i][:, None, :]
    tr, ti = eR[t_i][:, None, :], eI[t_i][:, None, :]
    h_re, h_im = hr * c_ - hi * s_, hr * s_ + hi * c_
    t_re, t_im = tr * c_ - ti * s_, tr * s_ + ti * c_
    rr, ri = rR[r_i], rI[r_i]

    def soft(lg):
        e = np.exp(lg - lg.max(axis=1, keepdims=True))
        return (e / e.sum(axis=1, keepdims=True))[..., None]

    a_r = soft(np.einsum("bd,bwd->bw", rr, h_re))
    a_i = soft(np.einsum("bd,bwd->bw", ri, h_im))
    b_r = soft(np.einsum("bd,bwd->bw", rr, t_re))
    b_i = soft(np.einsum("bd,bwd->bw", ri, t_im))
    y_r = (a_r * h_re).sum(1)
    y_i = (a_i * h_im).sum(1)
    z_r = (b_r * t_re).sum(1)
    z_i = (b_i * t_im).sum(1)
    return (np.abs(y_r + rr - z_r).sum(1)
            + np.abs(y_i + ri + z_i).sum(1)).astype(np.float32)


def build_nc():
    nc = bacc.Bacc(
        "TRN2",
        target_bir_lowering=False,
        debug=False,
        enable_asserts=False,
        num_devices=N_CORES,
    )
    ins = dict(
        embEp=nc.dram_tensor("embEp", [N_ENTITY, DD], BF16,
                             kind="ExternalInput").ap(),
        embRp=nc.dram_tensor("embRp", [N_RELATION, DD], BF16,
                             kind="ExternalInput").ap(),
        csF=nc.dram_tensor("csF", [P, IC * DD], BF16,
                           kind="ExternalInput").ap(),
        csT=nc.dram_tensor("csT", [P, JC * NI], BF16,
                           kind="ExternalInput").ap(),
        maskT=nc.dram_tensor("maskT", [P, IC * BL], BF16,
                             kind="ExternalInput").ap(),
        ht_idx=nc.dram_tensor("ht_idx", [P, T * 2], I32,
                              kind="ExternalInput").ap(),
        r_idx=nc.dram_tensor("r_idx", [P, T], I32,
                             kind="ExternalInput").ap(),
    )
    outs = dict(
        out=nc.dram_tensor("out", [P, T], F32, kind="ExternalOutput").ap(),
    )
    with tile.TileContext(nc) as tc:
        _emit(tc, outs, ins)
    nc.compile()
    return nc


_NC_CACHE = {}


def kernel(h_i, t_i, r_i, d_i, emb_E_real, emb_E_img, emb_R_real, emb_R_img,
           time_table, _want_results=False, _trace=False):
    in_maps, perms, fallback = _host_prep(
        h_i, t_i, r_i, d_i, emb_E_real, emb_E_img, emb_R_real, emb_R_img,
        time_table)
    if "nc" not in _NC_CACHE:
        _NC_CACHE["nc"] = build_nc()
    nc = _NC_CACHE["nc"]
    res = run_bass_kernel_spmd(
        nc, in_maps, core_ids=list(range(N_CORES)), trace=_trace)
    out = np.empty((B,), np.float32)
    for core in range(N_CORES):
        o = np.asarray(res.results[core]["out"]).T.reshape(BL)  # sorted order
        out[core * BL + perms[core]] = o
    if len(fallback):
        f = np.asarray(fallback)
        out[f] = _reference_np(
            np.asarray(h_i)[f], np.asarray(t_i)[f], np.asarray(r_i)[f],
            np.asarray(d_i)[f], np.asarray(emb_E_real),
            np.asarray(emb_E_img), np.asarray(emb_R_real),
            np.asarray(emb_R_img), np.asarray(time_table, dtype=np.float32))
    if _want_results:
        return out, res
    return out
